# revision 2
# baseline (speedup 1.0000x reference)
"""Sharded causal-attention kernel for TRN2 (8 cores), fp8-DoubleRow design.

Problem: x[4,2048,1024], W[2048,1024]:
  kv = x @ W.T ; K,V = split(kv) ; out = x + softmax(x@K.T + causal) @ V

Phase A (proj): core i (b=i//2, h=i%2) computes kv rows [h*1024:(h+1)*1024)
of batch b: K^T-half [1024, 1024] (f32r matmuls) and V-half [1024, 1024]
(3-product hi/lo fp8 DoubleRow matmuls, 4x PE throughput).

Phase B (attn): core i handles q-tiles {2j+h}; slot j spans L=256(j+1) keys.
Scores in f32r (SC_MODE="f32r") or 3-product hi/lo fp8 DR ("fp8").
Softmax reads PSUM directly (per-span DVE max partials, ACT exp->bf16 with
accumulated row sums). attn@V uses fp8 DR with attn and V both hi/lo split
(3 products). Residual add + dtype marshalling happen on host.
"""
import numpy as np
import ml_dtypes

import concourse.bass as bass
import concourse.tile as tile
from concourse import bacc, mybir
from concourse.bass_utils import run_bass_kernel_spmd

F8 = ml_dtypes.float8_e4m3
BF = ml_dtypes.bfloat16
F32 = np.float32
B, S, D = 4, 2048, 1024
NCORES = 8
P = 128
NDP = D // P
NSLOT = 8
NEG = -1e30
DR = mybir.MatmulPerfMode.DoubleRow
SLOT_ORDER = [1, 2, 3, 4, 5, 6, 7, 0]


def f8split(a):
    hi = np.asarray(a, F32).astype(F8)
    lo = (np.asarray(a, F32) - hi.astype(F32)).astype(F8)
    return hi, lo


# ---------------------------------------------------------------- kernel A
def build_proj(kt_bf16=False):
    nc = bacc.Bacc("TRN2", target_bir_lowering=False, debug=False,
                   num_devices=NCORES)
    f8, f32, bf = mybir.dt.float8e4, mybir.dt.float32, mybir.dt.bfloat16
    f32r = mybir.dt.float32r
    f16 = mybir.dt.float16
    xt_in = nc.dram_tensor("xt", [D, 1024], f16, kind="ExternalInput").ap()
    wtk_in = nc.dram_tensor("wtk", [D, 1024], f16, kind="ExternalInput").ap()
    xth_in = nc.dram_tensor("xth", [D, 1024], f8, kind="ExternalInput").ap()
    xtl_in = nc.dram_tensor("xtl", [D, 1024], f8, kind="ExternalInput").ap()
    wvh_in = nc.dram_tensor("wvh", [D, 1024], f8, kind="ExternalInput").ap()
    wvl_in = nc.dram_tensor("wvl", [D, 1024], f8, kind="ExternalInput").ap()
    kdt = f16
    kt_out = nc.dram_tensor("kt", [D, 1024], kdt, kind="ExternalOutput").ap()
    v_out = nc.dram_tensor("v", [1024, D], bf, kind="ExternalOutput").ap()

    xtr = xt_in.rearrange("(dp p) s -> p dp s", p=P)
    wkr = wtk_in.rearrange("(dp p) e -> p dp e", p=P)
    xhr = xth_in.rearrange("(dp p) s -> p dp s", p=P)
    xlr = xtl_in.rearrange("(dp p) s -> p dp s", p=P)
    wvhr = wvh_in.rearrange("(dp p) e -> p dp e", p=P)
    wvlr = wvl_in.rearrange("(dp p) e -> p dp e", p=P)
    ktr = kt_out.rearrange("(dt p) s -> p dt s", p=P)
    vr = v_out.rearrange("(st p) e -> p st e", p=P)

    with tile.TileContext(nc) as tc:
        with (
            tc.tile_pool(name="res", bufs=1) as res,
            tc.tile_pool(name="ob", bufs=4) as ob,
            tc.tile_pool(name="ps", bufs=1, space="PSUM") as psp,
        ):
            wk = res.tile([P, NDP, 1024], f16, tag="wk")
            xt = res.tile([P, NDP, 1024], f16, tag="xt")
            xh = res.tile([P, NDP, 1024], f8, tag="xh")
            xl = res.tile([P, NDP, 1024], f8, tag="xl")
            wvh = res.tile([P, NDP, 1024], f8, tag="wvh")
            wvl = res.tile([P, NDP, 1024], f8, tag="wvl")
            # K inputs first: fine-grained for dp0-1 (startup), pairs after
            nc.sync.dma_start(wk[:, 0, 0:128], wkr[:, 0, 0:128])
            nc.sync.dma_start(xt[:, 0, 0:512], xtr[:, 0, 0:512])
            nc.sync.dma_start(wk[:, 0, 128:1024], wkr[:, 0, 128:1024])
            nc.sync.dma_start(wk[:, 1, :], wkr[:, 1, :])
            nc.sync.dma_start(xt[:, 1, 0:512], xtr[:, 1, 0:512])
            for dp in range(2, NDP, 2):
                sl = slice(dp, dp + 2)
                nc.sync.dma_start(wk[:, sl, :], wkr[:, sl, :])
                nc.sync.dma_start(xt[:, sl, 0:512], xtr[:, sl, 0:512])
            for dp in range(0, NDP, 2):
                sl = slice(dp, dp + 2)
                nc.sync.dma_start(xt[:, sl, 512:1024], xtr[:, sl, 512:1024])
            for dp in range(0, NDP, 4):
                sl = slice(dp, dp + 4)
                nc.sync.dma_start(xh[:, sl, :], xhr[:, sl, :])
                nc.sync.dma_start(xl[:, sl, :], xlr[:, sl, :])
                nc.sync.dma_start(wvh[:, sl, :], wvhr[:, sl, :])
                nc.sync.dma_start(wvl[:, sl, :], wvlr[:, sl, :])

            # K^T[dt,span] = sum_dp wk[dp,dt].T @ xt[dp,span]  (f32r)
            # dp-outer order: every arriving dp chunk feeds 8 open psums
            for span in range(2):
                ss = bass.ts(span, 512)
                pss = [psp.tile([P, 512], f32, tag=f"ps{dt}",
                                name=f"ps{span}_{dt}") for dt in range(NDP)]
                if span == 0:
                    # DMA-paced phase: dp-outer, every chunk feeds 8 psums
                    order = [(dp, dt) for dp in range(NDP)
                             for dt in range(NDP)]
                else:
                    # compute-paced: dt-outer staggers the psum copies
                    order = [(dp, dt) for dt in range(NDP)
                             for dp in range(NDP)]
                for dp, dt in order:
                    es = slice(dt * P, (dt + 1) * P)
                    nc.tensor.matmul(pss[dt][:], wk[:, dp, es],
                                     xt[:, dp, ss],
                                     start=(dp == 0), stop=(dp == NDP - 1))
                    if dp == NDP - 1:
                        ko = ob.tile([P, 512], kdt, tag="ko")
                        nc.vector.tensor_copy(ko[:], pss[dt][:])
                        nc.gpsimd.dma_start(ktr[:, dt, ss], ko[:])
            # V[st,espan] = sum_dp xt[dp,st].T @ wv[dp,espan]  (fp8 DR 3-prod)
            prods = ((xh, wvh), (xh, wvl), (xl, wvh))
            nmm = 4 * len(prods)
            for st in range(NDP):
                qs = slice(st * P, (st + 1) * P)
                vo = ob.tile([P, 1024], bf, tag="vo")
                last = st == NDP - 1
                for espan in range(2):
                    ss = bass.ts(espan, 512)
                    ps = psp.tile([P, 512], f32,
                                  tag=f"ps{(st * 2 + espan) % NDP}",
                                  name=f"psv{st}_{espan}")
                    n = 0
                    for c in range(4):
                        sl = slice(2 * c, 2 * c + 2)
                        for lh, rh in prods:
                            nc.tensor.matmul(
                                ps[:], lh[:, sl, qs], rh[:, sl, ss],
                                start=(n == 0), stop=(n == nmm - 1),
                                perf_mode=DR)
                            n += 1
                    nc.scalar.mul(vo[:, ss], ps[:], 0.0625)
                    if last:
                        nc.sync.dma_start(vr[:, st, ss], vo[:, ss])
                if not last:
                    nc.gpsimd.dma_start(vr[:, st, :], vo[:])
    nc.compile()
    return nc


# ---------------------------------------------------------------- kernel B
def build_attn(sc_mode="f32r"):
    nc = bacc.Bacc("TRN2", target_bir_lowering=False, debug=False,
                   num_devices=NCORES)
    f8, f32, bf = mybir.dt.float8e4, mybir.dt.float32, mybir.dt.bfloat16
    f32r = mybir.dt.float32r
    f16 = mybir.dt.float16
    if sc_mode == "f32r":
        kt_in = [nc.dram_tensor("kt", [D, S], f16,
                                kind="ExternalInput").ap()]
        xtq_in = [nc.dram_tensor("xtq", [D, 1024], f16,
                                 kind="ExternalInput").ap()]
        sdt = f16
    else:
        kt_in = [nc.dram_tensor(n, [D, S], f8, kind="ExternalInput").ap()
                 for n in ("kth", "ktl")]
        xtq_in = [nc.dram_tensor(n, [D, 1024], f8,
                                 kind="ExternalInput").ap()
                  for n in ("xtqh", "xtql")]
        sdt = f8
    vh_in = nc.dram_tensor("vh", [S, D], f8, kind="ExternalInput").ap()
    vl_in = nc.dram_tensor("vl", [S, D], f8, kind="ExternalInput").ap()
    mask = nc.dram_tensor("mask", [NSLOT, P, 256], bf,
                          kind="ExternalInput").ap()
    ident = nc.dram_tensor("ident", [P, P], bf, kind="ExternalInput").ap()
    out = nc.dram_tensor("out", [1024, D], bf, kind="ExternalOutput").ap()

    ktr = [t.rearrange("(dp p) s -> p dp s", p=P) for t in kt_in]
    xtqr = [t.rearrange("(dp p) q -> p dp q", p=P) for t in xtq_in]
    vhr = vh_in.rearrange("(kt p) e -> p kt e", p=P)
    vlr = vl_in.rearrange("(kt p) e -> p kt e", p=P)
    outr = out.rearrange("(j p) e -> p j e", p=P)
    maskr = mask.rearrange("j p m -> p j m")

    GW_WARM = 8
    with tile.TileContext(nc) as tc:
        with (
            tc.tile_pool(name="kres", bufs=1) as kres,
            tc.tile_pool(name="vres", bufs=1) as vres,
            tc.tile_pool(name="xres", bufs=1) as xres,
            tc.tile_pool(name="cons", bufs=1) as cons,
            tc.tile_pool(name="sm", bufs=2) as smp,
            tc.tile_pool(name="at", bufs=2) as atp,
            tc.tile_pool(name="st", bufs=10) as stp,
            tc.tile_pool(name="io", bufs=2) as iop,
            tc.tile_pool(name="ps_s", bufs=4, space="PSUM") as ps_s,
            tc.tile_pool(name="ps_t", bufs=2, space="PSUM") as ps_t,
            tc.tile_pool(name="ps_o", bufs=1, space="PSUM") as ps_o,
        ):
            nk = len(kt_in)
            kk = [kres.tile([P, NDP, S], sdt, tag=f"k{i}", name=f"k{i}")
                  for i in range(nk)]
            xx = [xres.tile([P, NDP, 1024], sdt, tag=f"x{i}", name=f"x{i}")
                  for i in range(nk)]
            vv = [vres.tile([P, S // P, D], f8, tag=f"v{i}", name=f"v{i}")
                  for i in range(2)]
            msk = cons.tile([P, NSLOT, 256], bf, tag="msk")
            idt = cons.tile([P, P], bf, tag="idt")
            # loads ordered by slot need (SLOT_ORDER = 1,2,..,7,0):
            # slot1: xtq[128:256] + kt c0; slot2: xtq[256:512] + kt c1; ...
            for i in range(nk):
                nc.sync.dma_start(xx[i][:, :, 128:256],
                                  xtqr[i][:, :, 128:256])
            for g in range(4):
                dsl = slice(2 * g, 2 * g + 2)
                for i in range(nk):
                    nc.sync.dma_start(kk[i][:, dsl, 0:512],
                                      ktr[i][:, dsl, 0:512])
            nc.sync.dma_start(idt[:], ident[:])
            nc.sync.dma_start(msk[:, 0:4, :], maskr[:, 0:4, :])
            gs = slice(0, 2)
            nc.sync.dma_start(vv[0][:, gs, :], vhr[:, gs, :])
            nc.sync.dma_start(vv[1][:, gs, :], vlr[:, gs, :])
            for i in range(nk):
                nc.sync.dma_start(xx[i][:, :, 256:512],
                                  xtqr[i][:, :, 256:512])
            gs = slice(2, 4)
            nc.sync.dma_start(vv[0][:, gs, :], vhr[:, gs, :])
            nc.sync.dma_start(vv[1][:, gs, :], vlr[:, gs, :])
            nc.sync.dma_start(msk[:, 4:8, :], maskr[:, 4:8, :])
            for c in range(1, 4):
                cs = slice(c * 512, (c + 1) * 512)
                for i in range(nk):
                    nc.sync.dma_start(kk[i][:, :, cs], ktr[i][:, :, cs])
                qsl = slice((c + 1) * 256, (c + 2) * 256)
                if qsl.stop <= 1024:
                    for i in range(nk):
                        nc.sync.dma_start(xx[i][:, :, qsl],
                                          xtqr[i][:, :, qsl])
                gs = slice(c * 4, (c + 1) * 4)
                nc.sync.dma_start(vv[0][:, gs, :], vhr[:, gs, :])
                nc.sync.dma_start(vv[1][:, gs, :], vlr[:, gs, :])
            for i in range(nk):
                nc.sync.dma_start(xx[i][:, :, 0:128], xtqr[i][:, :, 0:128])

            # warm the PE p-state during the initial DMA wait
            for wi in range(30):
                ptw = ps_t.tile([P, GW_WARM, P], bf, tag="pt",
                                name=f"warm{wi}")
                nc.tensor.transpose(ptw[:, 0, :], idt[:], idt[:])

            if sc_mode == "f32r":
                sc_prods = ((xx[0], kk[0]),)
            else:
                sc_prods = ((xx[0], kk[0]), (xx[0], kk[1]), (xx[1], kk[0]))

            def emit_scores(j):
                L = 256 * (j + 1)
                ns = (L + 511) // 512
                qs = slice(j * P, (j + 1) * P)
                spans = []
                nmp = []
                sc = None
                if ns >= 3:
                    sc = smp.tile([P, 1024], f32, tag="sc", name=f"sc{j}")
                for si in range(ns):
                    c0 = si * 512
                    cw = min(512, L - c0)
                    ps = ps_s.tile([P, 512], f32, tag="ps",
                                   name=f"ps{j}_{si}")
                    if sc_mode == "f32r":
                        for dp in range(NDP):
                            nc.tensor.matmul(
                                ps[:, 0:cw], xx[0][:, dp, qs],
                                kk[0][:, dp, c0:c0 + cw],
                                start=(dp == 0), stop=(dp == NDP - 1))
                    else:
                        n = 0
                        nmm = 4 * len(sc_prods)
                        for c in range(4):
                            sl = slice(2 * c, 2 * c + 2)
                            for lh, rh in sc_prods:
                                nc.tensor.matmul(
                                    ps[:, 0:cw], lh[:, sl, qs],
                                    rh[:, sl, c0:c0 + cw],
                                    start=(n == 0), stop=(n == nmm - 1),
                                    perf_mode=DR)
                                n += 1
                    if c0 + cw == L:
                        nc.vector.tensor_tensor(
                            out=ps[:, cw - 256:cw], in0=ps[:, cw - 256:cw],
                            in1=msk[:, j, :], op=mybir.AluOpType.add)
                    nm = stp.tile([P, 1], f32, tag=f"nm{si}",
                                  name=f"nm{j}_{si}")
                    nc.vector.tensor_reduce(
                        nm[:], ps[:, 0:cw], axis=mybir.AxisListType.X,
                        op=mybir.AluOpType.max, negate=True)
                    nmp.append(nm)
                    # free psum early: bounce all but the last 2 spans
                    if si < ns - 2:
                        nc.vector.tensor_copy(sc[:, c0:c0 + cw], ps[:, 0:cw])
                        spans.append((sc, c0, cw, True))
                    else:
                        spans.append((ps, c0, cw, False))
                return (j, spans, nmp)

            def emit_tail(state):
                j, spans, nmp = state
                L = 256 * (j + 1)
                nkt = L // P
                while len(nmp) > 1:
                    nc.vector.tensor_tensor(
                        out=nmp[0][:], in0=nmp[0][:], in1=nmp[-1][:],
                        op=mybir.AluOpType.min)
                    nmp.pop()
                nmax = nmp[0]
                attn = smp.tile([P, L], bf, tag="attn", name=f"attn{j}")
                rsp = []
                for src_, c0, cw, bounced in spans:
                    r_ = stp.tile([P, 1], f32, tag=f"rs{c0 // 512}",
                                  name=f"rs{j}_{c0 // 512}")
                    src_ap = src_[:, c0:c0 + cw] if bounced else src_[:, 0:cw]
                    nc.scalar.activation(
                        attn[:, c0:c0 + cw], src_ap,
                        mybir.ActivationFunctionType.Exp,
                        bias=nmax[:], scale=1.0, accum_out=r_[:])
                    rsp.append(r_)
                while len(rsp) > 1:
                    nc.vector.tensor_tensor(
                        out=rsp[0][:], in0=rsp[0][:], in1=rsp[-1][:],
                        op=mybir.AluOpType.add)
                    rsp.pop()
                rs2 = stp.tile([P, 1], f32, tag="rs2", name=f"rs2{j}")
                nc.vector.tensor_scalar_mul(rs2[:], rsp[0][:], 16.0)
                rcp = stp.tile([P, 1], f32, tag="rcp", name=f"rcp{j}")
                nc.vector.reciprocal(rcp[:], rs2[:])
                # groups of 8 k-tiles: transpose (bf16 PE) -> fp8 hi (ACT)
                # + lo (DVE); attnV trails one group behind the transposes
                ath = atp.tile([P, 16, P], f8, tag="ath", name=f"ath{j}")
                atl = atp.tile([P, 16, P], f8, tag="atl", name=f"atl{j}")
                po = ps_o.tile([P, D], f32, tag="po", name=f"po{j}")
                npair = nkt // 2
                av_prods = ((ath, vv[0]), (ath, vv[1]), (atl, vv[0]))
                GW = 8
                groups = list(range(0, nkt, GW))

                def emit_tr(g0):
                    gw = min(GW, nkt - g0)
                    pt = ps_t.tile([P, GW, P], bf, tag="pt",
                                   name=f"pt{j}_{g0}")
                    for c in range(gw):
                        kt_i = g0 + c
                        nc.tensor.transpose(
                            pt[:, c, :], attn[:, kt_i * P:(kt_i + 1) * P],
                            idt[:])
                    gsl = slice(g0, g0 + gw)
                    nc.scalar.copy(ath[:, gsl, :], pt[:, 0:gw, :])
                    nc.vector.tensor_tensor(
                        out=atl[:, gsl, :], in0=pt[:, 0:gw, :],
                        in1=ath[:, gsl, :], op=mybir.AluOpType.subtract)

                def emit_av(g0):
                    gw = min(GW, nkt - g0)
                    for t in range(g0 // 2, (g0 + gw) // 2):
                        sl = slice(2 * t, 2 * t + 2)
                        for espan in range(2):
                            es = bass.ts(espan, 512)
                            for ia, (a_, v_) in enumerate(av_prods):
                                nc.tensor.matmul(
                                    po[:, es], a_[:, sl, :], v_[:, sl, es],
                                    start=(t == 0 and ia == 0),
                                    stop=(t == npair - 1 and ia == 2),
                                    perf_mode=DR)

                emit_tr(groups[0])
                for gi in range(1, len(groups)):
                    emit_tr(groups[gi])
                    emit_av(groups[gi - 1])
                emit_av(groups[-1])
                ot = iop.tile([P, D], bf, tag="ot", name=f"ot{j}")
                if j == SLOT_ORDER[-1]:
                    nc.scalar.mul(ot[:, 0:512], po[:, 0:512], rcp[:])
                    nc.sync.dma_start(outr[:, j, 0:512], ot[:, 0:512])
                    nc.scalar.mul(ot[:, 512:1024], po[:, 512:1024], rcp[:])
                    nc.sync.dma_start(outr[:, j, 512:1024], ot[:, 512:1024])
                else:
                    nc.scalar.mul(ot[:], po[:], rcp[:])
                    nc.gpsimd.dma_start(outr[:, j, :], ot[:])

            # software pipeline: slot j+1 scores issue before slot j's tail
            pending = None
            for j in SLOT_ORDER:
                st_ = emit_scores(j)
                if pending is not None:
                    emit_tail(pending)
                pending = st_
            emit_tail(pending)
    nc.compile()
    return nc


# ------------------------------------------------------------- host glue
def proj_in_maps(x, W):
    wtk = np.ascontiguousarray(W[:D].T).astype(np.float16)
    # scale into e4m3's sweet range: W*64 (~1.3), x*4 (~4); V comes out
    # as 256*V in psum, stored as 16*V (ACT copy scale 1/16)
    wvh, wvl = f8split(np.ascontiguousarray(W[D:].T) * 64.0)
    maps = []
    for i in range(NCORES):
        b, h = divmod(i, 2)
        xt = np.ascontiguousarray(x[b, h * 1024:(h + 1) * 1024, :].T)
        xth, xtl = f8split(xt * 4.0)
        xt = xt.astype(np.float16)
        maps.append({"xt": xt, "wtk": wtk, "xth": xth, "xtl": xtl,
                     "wvh": wvh, "wvl": wvl})
    return maps


def make_masks():
    tri = np.triu(np.full((P, P), NEG, dtype=F32), 1)
    masks = []
    for h in range(2):
        m = np.zeros((NSLOT, P, 256), F32)
        for j in range(NSLOT):
            if h == 1:
                m[j, :, 128:] = tri
            else:
                m[j, :, :128] = tri
                m[j, :, 128:] = NEG
        masks.append(m.astype(BF))
    return masks


def attn_in_maps(x, kt_full, v_full, sc_mode="f32r"):
    masks = make_masks()
    ident = np.eye(P, dtype=F32).astype(BF)
    maps = []
    ktcache = {}
    for i in range(NCORES):
        b, h = divmod(i, 2)
        qidx = [2 * j + h for j in range(NSLOT)]
        xt = x[b].T
        xtq = np.ascontiguousarray(
            np.concatenate([xt[:, t * P:(t + 1) * P] for t in qidx], axis=1))
        if b not in ktcache:
            vhb, vlb = f8split(v_full[b])
            if sc_mode == "f32r":
                ktcache[b] = ({"kt": np.ascontiguousarray(kt_full[b])
                               .astype(np.float16)},
                              vhb, vlb)
            else:
                kth, ktl = f8split(kt_full[b])
                ktcache[b] = ({"kth": kth, "ktl": ktl}, vhb, vlb)
        ktm, vhb, vlb = ktcache[b]
        m = {"vh": vhb, "vl": vlb, "mask": masks[h], "ident": ident}
        m.update(ktm)
        if sc_mode == "f32r":
            m["xtq"] = xtq.astype(np.float16)
        else:
            m["xtqh"], m["xtql"] = f8split(xtq)
        maps.append(m)
    return maps


def assemble_proj(results):
    kt = np.stack([np.concatenate(
        [np.asarray(results[2 * b]["kt"], np.float16),
         np.asarray(results[2 * b + 1]["kt"], np.float16)], axis=1)
        for b in range(B)])
    v = np.stack([np.concatenate(
        [np.asarray(results[2 * b]["v"], F32),
         np.asarray(results[2 * b + 1]["v"], F32)], axis=0)
        for b in range(B)])
    return kt, v


def assemble_out(x, results):
    out = np.empty((B, S, D), F32)
    for i in range(NCORES):
        b, h = divmod(i, 2)
        for j in range(NSLOT):
            t = 2 * j + h
            out[b, t * P:(t + 1) * P, :] = (
                x[b, t * P:(t + 1) * P, :]
                + np.asarray(results[i]["out"][j * P:(j + 1) * P], F32))
    return out


# ===================================================================
SC_MODE = "f32r"
_CACHE = {}


def _get_kernels():
    if "proj" not in _CACHE:
        _CACHE["proj"] = build_proj(kt_bf16=(SC_MODE != "f32r"))
        _CACHE["attn"] = build_attn(sc_mode=SC_MODE)
    return _CACHE["proj"], _CACHE["attn"]


def kernel(x, W):
    x = np.asarray(x, dtype=F32)
    W = np.asarray(W, dtype=F32)
    nc_proj, nc_attn = _get_kernels()

    mapsA = proj_in_maps(x, W)
    resA = run_bass_kernel_spmd(nc_proj, mapsA, list(range(NCORES))).results
    kt_full, v_full = assemble_proj(resA)

    mapsB = attn_in_maps(x, kt_full, v_full, SC_MODE)
    resB = run_bass_kernel_spmd(nc_attn, mapsB, list(range(NCORES))).results
    return assemble_out(x, resB)
